# revision 34
# baseline (speedup 1.0000x reference)
"""GATNet (2-layer GAT, 50000 nodes / 800000 edges) on 8 Trainium2 cores.

Strategy: dst-sharding with edge-gather pipelines. Each core owns N/8
destination nodes; edges are bucketed by (dst block of 128, src lo/hi table)
on the host, sorted by src, and padded with trailing -1 indices (the gather
ucode trims trailing negatives for free). Per-edge node records are fetched
with dma_gather (bf16 768B rows for layer 1, f32 256B rows for layer 2),
round-robined over the 4 SWDGE queues so descriptor generation pipelines
across Q7 core pairs.

Per 128-dst block: one-hot selector tiles S [slot, dst] are built ON-CHIP
from a compact per-slot dst-index stream (DVE is_equal vs an iota ruler),
transposed on PE for the al_dst expansion matmul; attention logits = gathered
al_src + S^T-matmul of al_dst; exp on ACT; per-edge message weighting on DVE;
segment-sum via S-matmul into PSUM (denominator rides as extra matmul
columns); finalize = div + ELU; layer-2 records built via transpose+matmul,
AllGather (1.6MB/core), then the same edge pipeline on 64-col f32 records
with a log_softmax finalize.

The index stream is identical for both layers (same edges, same node row
indexing) and lives in SBUF for the whole kernel.
"""

import sys
import numpy as np

sys.path.insert(0, "/opt/trn_rl_repo")

NCORES = 8
BLK = 128
TILE = 128
LO_LIM = 32768
HEADS, HID, OUT_CH = 8, 32, 16
F1 = HEADS * HID            # 256
R1_W = 384                  # layer-1 record row, bf16 (256 + 8 pad to 768B)
REC_W = 64                  # layer-2 record row, f32 (16 + 1 + 1 pad to 256B)
NEG_SLOPE = 0.2
DEN_EPS = 1e-30
GCAP = 8                    # tiles per dma_gather call (<=1024 idxs)
NO_TRIM = True              # debug: gather pad slots instead of -1 trim


# ---------------------------------------------------------------- planning

class _P:
    pass


def _plan(edge_index, n_nodes, lo_lim=LO_LIM):
    ndst = n_nodes // NCORES
    nblk = (ndst + BLK - 1) // BLK
    src = np.concatenate([edge_index[0], np.arange(n_nodes)]).astype(np.int64)
    dst = np.concatenate([edge_index[1], np.arange(n_nodes)]).astype(np.int64)
    owner = dst // ndst
    per_core = []
    cnt = np.zeros((NCORES, nblk, 2), np.int64)
    for k in range(NCORES):
        m = owner == k
        s_k = src[m]
        d_k = dst[m] - k * ndst
        b_k = d_k // BLK
        j_k = d_k % BLK
        kind = (s_k >= lo_lim).astype(np.int64)   # 0 = lo, 1 = hi
        order = np.lexsort((s_k, kind, b_k))
        s_k, b_k, j_k, kind = s_k[order], b_k[order], j_k[order], kind[order]
        per_core.append((s_k, b_k, j_k, kind))
        np.add.at(cnt, (k, b_k, kind), 1)
    T_lo = np.maximum(1, -(-cnt[:, :, 0].max(axis=0) // TILE))
    T_hi = np.maximum(1, -(-cnt[:, :, 1].max(axis=0) // TILE))
    T_all = T_lo + T_hi
    off = np.zeros(nblk + 1, np.int64)
    off[1:] = np.cumsum(T_all)
    TT = int(off[-1])

    pl = _P()
    pl.ndst, pl.nblk, pl.n_nodes, pl.lo_lim = ndst, nblk, n_nodes, lo_lim
    pl.T_lo, pl.T_hi, pl.off, pl.TT = T_lo, T_hi, off, TT
    pl.cores = []
    for k in range(NCORES):
        s_k, b_k, j_k, kind = per_core[k]
        slot_src = np.full(TT * TILE, -1, np.int64)
        slot_j = np.full(TT * TILE, 255, np.int64)
        # per (block, kind) segment: real edges first, then -1 pads
        seg_id = b_k * 2 + kind
        bounds = np.searchsorted(seg_id, np.arange(2 * nblk + 1))
        for b in range(nblk):
            for kd in range(2):
                lo_i, hi_i = bounds[b * 2 + kd], bounds[b * 2 + kd + 1]
                n = hi_i - lo_i
                if n == 0:
                    continue
                s0 = (off[b] + (0 if kd == 0 else T_lo[b])) * TILE
                vals = s_k[lo_i:hi_i] - (lo_lim if kd else 0)
                slot_src[s0:s0 + n] = vals
                slot_j[s0:s0 + n] = j_k[lo_i:hi_i]
        # no gather call may trim to zero descriptors: give the first slot of
        # every call range a harmless real index (j stays 255 -> no effect)
        for b in range(nblk):
            for kd in range(2):
                T = (T_lo if kd == 0 else T_hi)[b]
                s0 = (off[b] + (0 if kd == 0 else T_lo[b])) * TILE
                for t0 in range(0, int(T), GCAP):
                    p = s0 + t0 * TILE
                    if slot_src[p] < 0:
                        slot_src[p] = 0
        if NO_TRIM:
            slot_src[slot_src < 0] = 0
        # per-call valid-prefix counts (gather trims trailing -1 to this)
        cnts = []
        for b in range(nblk):
            for kd in range(2):
                T = (T_lo if kd == 0 else T_hi)[b]
                s0 = (off[b] + (0 if kd == 0 else T_lo[b])) * TILE
                for t0 in range(0, int(T), GCAP):
                    t1 = min(t0 + GCAP, int(T))
                    seg = slot_src[s0 + t0 * TILE:s0 + t1 * TILE]
                    valid = np.nonzero(seg >= 0)[0]
                    cnts.append(int(valid[-1]) + 1 if len(valid) else 0)
        cp = _P()
        cp.slot_src, cp.slot_j = slot_src, slot_j
        cp.call_cnt = np.array(cnts, np.int32)
        pl.cores.append(cp)
    pl.ncalls = len(pl.cores[0].call_cnt)
    return pl


def _build_streams(pl, k):
    cp = pl.cores[k]
    TT = pl.TT
    # idx stream, wrapped [16, TT*8] then replicated to 8 Q7 groups
    idx = np.tile(cp.slot_src.astype(np.int16).reshape(TT * 8, 16).T, (8, 1))
    jst = cp.slot_j.reshape(TT, TILE).T.astype(np.float32)     # [128, TT]
    cnt = np.tile(cp.call_cnt[None, :], (128, 1))              # [128, ncalls]
    return np.ascontiguousarray(idx), np.ascontiguousarray(jst), \
        np.ascontiguousarray(cnt)


# ---------------------------------------------------------------- program

def build_program(pl, want_debug=False):
    import concourse.bass as bass
    import concourse.bacc as bacc
    import concourse.tile as tile
    import concourse.mybir as mybir

    F32 = mybir.dt.float32
    BF16 = mybir.dt.bfloat16
    I16 = mybir.dt.int16
    AF = mybir.ActivationFunctionType
    ALU = mybir.AluOpType

    n_nodes = pl.n_nodes
    ndst, nblk = pl.ndst, pl.nblk
    lo_lim = pl.lo_lim
    hi_rows = n_nodes - lo_lim
    T_lo, T_hi, off, TT = pl.T_lo, pl.T_hi, pl.off, pl.TT
    TLO_MX = int(T_lo.max())
    THI_MX = int(T_hi.max())
    TAL_MX = int((T_lo + T_hi).max())
    NPAD = nblk * BLK

    nc = bacc.Bacc("TRN2", target_bir_lowering=False, debug=want_debug,
                   num_devices=NCORES, num_swdge_queues=4,
                   dynamic_dma_scratch_size=32768)
    qrr = [0]

    def next_q():
        q = qrr[0] % 4
        qrr[0] += 1
        return q

    # -------- inputs
    xT = nc.dram_tensor("xT", [128, n_nodes], BF16, kind="ExternalInput")
    xoT = nc.dram_tensor("xoT", [128, NPAD], BF16, kind="ExternalInput")
    W1e = nc.dram_tensor("W1e", [128, F1 + HEADS], BF16, kind="ExternalInput")
    Vd1 = nc.dram_tensor("Vd1", [128, HEADS], BF16, kind="ExternalInput")
    WV2 = nc.dram_tensor("WV2", [128, 2, 18], BF16, kind="ExternalInput")
    IDENT = nc.dram_tensor("IDENT", [128, 128], BF16, kind="ExternalInput")
    IOTA = nc.dram_tensor("IOTA", [128, 128], F32, kind="ExternalInput")
    B1R = nc.dram_tensor("B1R", [128, F1], BF16, kind="ExternalInput")
    B2R = nc.dram_tensor("B2R", [128, OUT_CH], F32, kind="ExternalInput")
    RECB = nc.dram_tensor("RECB", [128, 18], F32, kind="ExternalInput")
    IDX = nc.dram_tensor("IDX", [128, TT * 8], I16, kind="ExternalInput")
    JST = nc.dram_tensor("JST", [128, TT], F32, kind="ExternalInput")
    OUT = nc.dram_tensor("OUT", [ndst, OUT_CH], F32, kind="ExternalOutput")

    NCH = -(-n_nodes // 128)
    BIGC = 8

    with tile.TileContext(nc) as tc:
        with (
            tc.tile_pool(name="dram", bufs=1, space="DRAM") as dpool,
            tc.tile_pool(name="const", bufs=1) as cpool,
            tc.tile_pool(name="persist", bufs=1) as ppool,
            tc.tile_pool(name="node", bufs=3) as npool,
            tc.tile_pool(name="edge", bufs=2) as epool,
            tc.tile_pool(name="sel", bufs=2) as selpool,
            tc.tile_pool(name="small", bufs=3) as spool,
            tc.tile_pool(name="ps_e", bufs=2, space="PSUM") as ps_e,
            tc.tile_pool(name="ps_a", bufs=2, space="PSUM") as ps_a,
            tc.tile_pool(name="ps_m", bufs=2, space="PSUM") as ps_m,
            tc.tile_pool(name="ps_n", bufs=2, space="PSUM") as ps_n,
        ):
            R1 = dpool.tile([n_nodes, R1_W], BF16)
            REC = dpool.tile([ndst, REC_W], F32)
            R2 = dpool.tile([n_nodes, REC_W], F32, addr_space="Shared")

            # consts
            cW1e = cpool.tile([128, F1 + HEADS], BF16)
            cVd1 = cpool.tile([128, HEADS], BF16)
            cWV2 = cpool.tile([128, 2, 18], BF16)
            cID = cpool.tile([128, 128], BF16)
            cIOTA = cpool.tile([128, 128], F32)
            cB1 = cpool.tile([128, F1], BF16)
            cB2 = cpool.tile([128, OUT_CH], F32)
            cRB = cpool.tile([128, 18], F32)
            cZ = cpool.tile([128, F1], BF16)
            nc.vector.memset(cZ[:], 0.0)
            nc.sync.dma_start(cW1e[:], W1e[:])
            nc.sync.dma_start(cVd1[:], Vd1[:])
            nc.sync.dma_start(cWV2[:], WV2[:])
            nc.sync.dma_start(cID[:], IDENT[:])
            nc.sync.dma_start(cIOTA[:], IOTA[:])
            nc.sync.dma_start(cB1[:], B1R[:])
            nc.sync.dma_start(cB2[:], B2R[:])
            nc.sync.dma_start(cRB[:], RECB[:])

            cIDX = ppool.tile([128, TT * 8], I16)
            cJ = ppool.tile([128, TT], F32)
            nc.sync.dma_start(cIDX[:], IDX[:])
            nc.sync.dma_start(cJ[:], JST[:])

            alD1 = ppool.tile([128, nblk, HEADS], BF16)
            alD2 = ppool.tile([128, nblk, 1], BF16)

            # ---------------- N1: R1 node table (bf16 records)
            for c0 in range(0, NCH, BIGC):
                c1 = min(c0 + BIGC, NCH)
                ncols = min(n_nodes - c0 * 128, BIGC * 128)
                xt = npool.tile([128, BIGC * 128], BF16, tag="xt")
                nc.sync.dma_start(xt[:, 0:ncols], xT[:, c0 * 128:c0 * 128 + ncols])
                rw = npool.tile([128, BIGC, F1 + HEADS], BF16, tag="rw")
                for c in range(c0, c1):
                    m = min(128, n_nodes - c * 128)
                    pn = ps_n.tile([128, F1 + HEADS], F32, tag="pn",
                                   padded_shape=[128, 512])
                    nc.tensor.matmul(pn[0:m, :],
                                     xt[:, (c - c0) * 128:(c - c0) * 128 + m],
                                     cW1e[:], start=True, stop=True)
                    nc.scalar.copy(rw[0:m, c - c0, :], pn[0:m, :])
                nfull = (min(n_nodes, c1 * 128) - c0 * 128) // 128
                if nfull:
                    nc.sync.dma_start(
                        R1[c0 * 128:c0 * 128 + nfull * 128, 0:F1 + HEADS]
                        .rearrange("(g p) e -> p g e", p=128),
                        rw[:, 0:nfull, :])
                rem = min(n_nodes, c1 * 128) - c0 * 128 - nfull * 128
                if rem:
                    nc.sync.dma_start(
                        R1[c0 * 128 + nfull * 128:min(n_nodes, c1 * 128),
                           0:F1 + HEADS]
                        .rearrange("(g p) e -> p g e", p=rem),
                        rw[0:rem, nfull:nfull + 1, :])

            # ---------------- N2: own al_dst1
            xo = ppool.tile([128, NPAD], BF16)
            nc.sync.dma_start(xo[:], xoT[:])
            for b in range(nblk):
                po = ps_n.tile([128, HEADS], F32, tag="pn",
                               padded_shape=[128, 512])
                nc.tensor.matmul(po[:], xo[:, b * 128:(b + 1) * 128], cVd1[:],
                                 start=True, stop=True)
                nc.scalar.copy(alD1[:, b, :], po[:])

            # ---------------- shared edge-stage builder
            def edge_stage(layer, post_block=None):
                if layer == 1:
                    EW, EDT = R1_W, BF16
                    F, H = F1, HEADS
                    tbl_lo = R1[0:lo_lim, :]
                    tbl_hi = R1[lo_lim:n_nodes, :]
                    alD = alD1
                else:
                    EW, EDT = REC_W, F32
                    F, H = OUT_CH, 1
                    tbl_lo = R2[0:lo_lim, :]
                    tbl_hi = R2[lo_lim:n_nodes, :]
                    alD = alD2
                for b in range(nblk):
                    tl, th = int(T_lo[b]), int(T_hi[b])
                    ta = tl + th
                    ob = int(off[b])
                    gl = epool.tile([128, TLO_MX, EW], EDT, tag=f"gl{layer}")
                    gh = epool.tile([128, THI_MX, EW], EDT, tag=f"gh{layer}")
                    if b < 2:
                        nc.vector.memset(gl[:], 0.0)
                        nc.vector.memset(gh[:], 0.0)
                    for t0 in range(0, tl, GCAP):
                        t1 = min(t0 + GCAP, tl)
                        ni = (t1 - t0) * TILE
                        nc.gpsimd.dma_gather(
                            gl[:, t0:t1, :], tbl_lo,
                            cIDX[:, (ob + t0) * 8:(ob + t1) * 8], ni, ni,
                            EW, queue_num=next_q())
                    for t0 in range(0, th, GCAP):
                        t1 = min(t0 + GCAP, th)
                        ni = (t1 - t0) * TILE
                        nc.gpsimd.dma_gather(
                            gh[:, t0:t1, :], tbl_hi,
                            cIDX[:, (ob + tl + t0) * 8:(ob + tl + t1) * 8],
                            ni, ni, EW, queue_num=next_q())

                    # layer 2: compact gathered records to bf16 [slot, t, 18]
                    if layer == 2:
                        g2b = spool.tile([128, TAL_MX, 18], BF16, tag="g2b")
                        nc.vector.tensor_copy(g2b[:, 0:tl, :], gl[:, 0:tl, 0:18])
                        nc.vector.tensor_copy(g2b[:, tl:ta, :], gh[:, 0:th, 0:18])

                    # S tiles: one-hot [slot, t, dst] built on-chip
                    sS = selpool.tile([128, TAL_MX, 128], BF16, tag="sS")
                    nc.vector.tensor_tensor(
                        sS[:, 0:ta, :],
                        cJ[:, ob:ob + ta].unsqueeze(2).broadcast_to([128, ta, 128]),
                        cIOTA[:].unsqueeze(1).broadcast_to([128, ta, 128]),
                        op=ALU.is_equal)

                    # batched: 8x ST = S^T on PE -> one ACT copy -> expansions
                    pe = ps_e.tile([128, TAL_MX * HEADS], F32, tag="pe",
                                   padded_shape=[128, 512])
                    for tb in range(0, ta, 8):
                        te = min(tb + 8, ta)
                        ptr = ps_n.tile([128, 8, 128], BF16, tag="pn",
                                        padded_shape=[128, 8, 128])
                        for ti in range(tb, te):
                            nc.tensor.transpose(ptr[:, ti - tb, :],
                                                sS[:, ti, :], cID[:])
                        sTb = spool.tile([128, 8, 128], BF16, tag="sT")
                        nc.scalar.copy(sTb[:, 0:te - tb, :],
                                       ptr[:, 0:te - tb, :])
                        for ti in range(tb, te):
                            nc.tensor.matmul(pe[:, ti * H:(ti + 1) * H],
                                             sTb[:, ti - tb, :],
                                             alD[:, b, :],
                                             start=True, stop=True)
                    eD = spool.tile([128, TAL_MX * HEADS], BF16, tag="eD")
                    nc.scalar.copy(eD[:, 0:ta * H], pe[:, 0:ta * H])

                    # e = al_src + e_dst ; lrelu ; exp ; weight messages
                    def gv(lo):
                        if layer == 2:
                            return g2b[:, 0:tl, :] if lo else g2b[:, tl:ta, :]
                        return gl[:, 0:tl, :] if lo else gh[:, 0:th, :]

                    exw = selpool.tile([128, TAL_MX, F], BF16,
                                       tag=f"exw{layer}")
                    for lo in (True, False):
                        t = gv(lo)
                        tn = tl if lo else th
                        eoff = 0 if lo else tl * H
                        toff = 0 if lo else tl
                        ecols = t[:, :, F:F + H]
                        nc.vector.tensor_add(
                            ecols, ecols,
                            eD[:, eoff:eoff + tn * H].rearrange(
                                "p (t h) -> p t h", h=H))
                        nc.vector.scalar_tensor_tensor(
                            ecols, ecols, NEG_SLOPE, ecols,
                            op0=ALU.mult, op1=ALU.max)
                        nc.scalar.activation(ecols, ecols, AF.Exp)
                        C = F // H
                        # expand per-(slot,head) weights to F cols on ACT so
                        # the DVE multiply below runs contiguous at 2x rate
                        nc.scalar.copy(
                            exw[:, toff:toff + tn, :].rearrange(
                                "p t (h c) -> p t h c", c=C),
                            ecols.unsqueeze(3).broadcast_to([128, tn, H, C]))
                        nc.vector.tensor_mul(
                            t[:, :, 0:F], t[:, :, 0:F],
                            exw[:, toff:toff + tn, :])

                    # aggregation: psum [dst, F+H] accumulated over tiles
                    pa = ps_a.tile([128, F + H], F32, tag="pa",
                                   padded_shape=[128, 512])
                    for ti in range(ta):
                        lo = ti < tl
                        i = ti if lo else ti - tl
                        rhs = gv(lo)[:, i, 0:F + H]
                        nc.tensor.matmul(pa[:], sS[:, ti, :], rhs,
                                         start=(ti == 0), stop=(ti == ta - 1))

                    # finalize
                    den = spool.tile([128, H], F32, tag="den")
                    nc.scalar.activation(den[:], pa[:, F:F + H], AF.Copy,
                                         bias=DEN_EPS)
                    rden = spool.tile([128, H], F32, tag="rden")
                    nc.vector.reciprocal(rden[:], den[:])
                    m = min(BLK, ndst - b * BLK)
                    if layer == 1:
                        C = F // H
                        h2t = spool.tile([128, F], BF16, tag="h2t")
                        nc.vector.tensor_mul(
                            h2t[:].rearrange("p (h c) -> p h c", c=C),
                            pa[:, 0:F].rearrange("p (h c) -> p h c", c=C),
                            rden[:].unsqueeze(2).broadcast_to([128, H, C]))
                        nc.vector.tensor_add(h2t[:], h2t[:], cB1[:])
                        # ELU without the -1 (folded into record bias)
                        t1 = spool.tile([128, F], BF16, tag="t1")
                        nc.vector.tensor_tensor(t1[:], h2t[:], cZ[:],
                                                op=ALU.min)
                        nc.scalar.activation(t1[:], t1[:], AF.Exp)
                        nc.vector.tensor_tensor(h2t[:], h2t[:], cZ[:],
                                                op=ALU.max)
                        nc.vector.tensor_add(h2t[:], h2t[:], t1[:])
                        ptr = ps_m.tile([128, 2, 128], BF16, tag="m",
                                        padded_shape=[128, 2, 128])
                        nc.tensor.transpose(ptr[:, 0, :], h2t[:, 0:128], cID[:])
                        nc.tensor.transpose(ptr[:, 1, :], h2t[:, 128:256], cID[:])
                        h2T = spool.tile([128, 2, 128], BF16, tag="h2T")
                        nc.scalar.copy(h2T[:], ptr[:])
                        prc = ps_m.tile([128, 18], F32, tag="m",
                                        padded_shape=[128, 512])
                        nc.tensor.matmul(prc[:], h2T[:, 0, :], cWV2[:, 0, :],
                                         start=True, stop=False)
                        nc.tensor.matmul(prc[:], h2T[:, 1, :], cWV2[:, 1, :],
                                         start=False, stop=True)
                        rec = spool.tile([128, 18], F32, tag="rec")
                        nc.vector.tensor_add(rec[:], prc[:], cRB[:])
                        nc.scalar.copy(alD2[:, b, :], rec[:, 17:18])
                        nc.sync.dma_start(REC[b * BLK:b * BLK + m, 0:18],
                                          rec[0:m, :])
                    else:
                        v = spool.tile([128, OUT_CH], F32, tag="v")
                        nc.vector.tensor_mul(
                            v[:], pa[:, 0:OUT_CH],
                            rden[:].broadcast_to([128, OUT_CH]))
                        nc.vector.tensor_add(v[:], v[:], cB2[:])
                        mx = spool.tile([128, 1], F32, tag="mx")
                        nc.vector.tensor_reduce(mx[:], v[:], op=ALU.max,
                                                axis=mybir.AxisListType.X)
                        nc.vector.tensor_sub(
                            v[:], v[:], mx[:].broadcast_to([128, OUT_CH]))
                        ex = spool.tile([128, OUT_CH], F32, tag="exf")
                        sm = spool.tile([128, 1], F32, tag="sm")
                        nc.scalar.activation(ex[:], v[:], AF.Exp,
                                             accum_out=sm[:])
                        lns = spool.tile([128, 1], F32, tag="lns")
                        nc.scalar.activation(lns[:], sm[:], AF.Ln)
                        nc.vector.tensor_sub(
                            v[:], v[:], lns[:].broadcast_to([128, OUT_CH]))
                        nc.sync.dma_start(OUT[b * BLK:b * BLK + m, :],
                                          v[0:m, :])
                    if post_block is not None:
                        post_block(b)

            # ---------------- E1, AllGather, E2
            edge_stage(1)
            nc.gpsimd.collective_compute(
                "AllGather", mybir.AluOpType.bypass,
                replica_groups=[list(range(NCORES))],
                ins=[REC.opt()], outs=[R2.opt()])
            edge_stage(2)

    nc.compile()
    return nc


# ---------------------------------------------------------------- host prep

def _host_inputs(pl, inputs):
    x = np.asarray(inputs["x"], np.float32)
    W1 = np.asarray(inputs["W1"], np.float32)
    a_s1 = np.asarray(inputs["a_src1"], np.float32)
    a_d1 = np.asarray(inputs["a_dst1"], np.float32)
    b1 = np.asarray(inputs["b1"], np.float32)
    W2 = np.asarray(inputs["W2"], np.float32)
    a_s2 = np.asarray(inputs["a_src2"], np.float32)
    a_d2 = np.asarray(inputs["a_dst2"], np.float32)
    b2 = np.asarray(inputs["b2"], np.float32)
    n_nodes, ndst, nblk = pl.n_nodes, pl.ndst, pl.nblk
    NPAD = nblk * BLK

    A_s1 = np.zeros((F1, HEADS), np.float32)
    A_d1 = np.zeros((F1, HEADS), np.float32)
    for h in range(HEADS):
        A_s1[h * HID:(h + 1) * HID, h] = a_s1[h]
        A_d1[h * HID:(h + 1) * HID, h] = a_d1[h]
    V_s1 = (W1 @ A_s1).astype(np.float32)
    V_d1 = (W1 @ A_d1).astype(np.float32)
    V_s2 = (W2 @ a_s2[0]).astype(np.float32)
    V_d2 = (W2 @ a_d2[0]).astype(np.float32)
    WV2 = np.concatenate([W2, V_s2[:, None], V_d2[:, None]], axis=1)  # [256,18]
    RECB = -WV2.sum(axis=0, keepdims=True)          # [1, 18] (the ELU -1 fold)

    import ml_dtypes
    BF = ml_dtypes.bfloat16
    iota = np.tile(np.arange(128, dtype=np.float32)[None, :], (128, 1))
    common = {
        "xT": np.ascontiguousarray(x.T.astype(BF)),
        "W1e": np.ascontiguousarray(
            np.concatenate([W1, V_s1], axis=1).astype(BF)),
        "Vd1": np.ascontiguousarray(V_d1.astype(BF)),
        "WV2": np.ascontiguousarray(
            WV2.reshape(2, 128, 18).transpose(1, 0, 2).astype(BF)),
        "IDENT": np.eye(128, dtype=np.float32).astype(BF),
        "IOTA": np.ascontiguousarray(iota),
        "B1R": np.tile(b1[None, :], (128, 1)).astype(BF),
        "B2R": np.tile(b2[None, :], (128, 1)).astype(np.float32),
        "RECB": np.tile(RECB, (128, 1)).astype(np.float32),
    }
    in_maps = []
    for k in range(NCORES):
        idx, jst, cnt = _build_streams(pl, k)
        xo = np.zeros((NPAD, 128), np.float32)
        xo[0:ndst] = x[k * ndst:(k + 1) * ndst]
        m = dict(common)
        m["xoT"] = np.ascontiguousarray(xo.T.astype(BF))
        m["IDX"] = idx
        m["JST"] = jst
        in_maps.append(m)
    return in_maps


# ---------------------------------------------------------------- entry

def _run(inputs, trace=False, **kw):
    from concourse.bass_utils import run_bass_kernel_spmd

    edge_index = np.asarray(inputs["edge_index"])
    n_nodes = int(np.asarray(inputs["x"]).shape[0])
    pl = _plan(edge_index, n_nodes)
    nc = build_program(pl)
    in_maps = _host_inputs(pl, inputs)
    res = run_bass_kernel_spmd(nc, in_maps, list(range(NCORES)),
                               trace=trace, **kw)
    out = np.concatenate([res.results[k]["OUT"] for k in range(NCORES)], axis=0)
    return out.astype(np.float32), res


def kernel(**inputs):
    out, _ = _run(inputs)
    return out


# revision 35
# speedup vs baseline: 1.0223x; 1.0223x over previous
"""GATNet (2-layer GAT, 50000 nodes / 800000 edges) on 8 Trainium2 cores.

Strategy: dst-sharding with edge-gather pipelines. Each core owns N/8
destination nodes; edges are bucketed by (dst block of 128, src lo/hi table)
on the host, sorted by src, and padded with trailing -1 indices (the gather
ucode trims trailing negatives for free). Per-edge node records are fetched
with dma_gather (bf16 768B rows for layer 1, f32 256B rows for layer 2),
round-robined over the 4 SWDGE queues so descriptor generation pipelines
across Q7 core pairs.

Per 128-dst block: one-hot selector tiles S [slot, dst] are built ON-CHIP
from a compact per-slot dst-index stream (DVE is_equal vs an iota ruler),
transposed on PE for the al_dst expansion matmul; attention logits = gathered
al_src + S^T-matmul of al_dst; exp on ACT; per-edge message weighting on DVE;
segment-sum via S-matmul into PSUM (denominator rides as extra matmul
columns); finalize = div + ELU; layer-2 records built via transpose+matmul,
AllGather (1.6MB/core), then the same edge pipeline on 64-col f32 records
with a log_softmax finalize.

The index stream is identical for both layers (same edges, same node row
indexing) and lives in SBUF for the whole kernel.
"""

import sys
import numpy as np

sys.path.insert(0, "/opt/trn_rl_repo")

NCORES = 8
BLK = 128
TILE = 128
LO_LIM = 32768
HEADS, HID, OUT_CH = 8, 32, 16
F1 = HEADS * HID            # 256
R1_W = 384                  # layer-1 record row, bf16 (256 + 8 pad to 768B)
REC_W = 64                  # layer-2 record row, f32 (16 + 1 + 1 pad to 256B)
NEG_SLOPE = 0.2
DEN_EPS = 1e-30
GCAP = 8                    # tiles per dma_gather call (<=1024 idxs)
NO_TRIM = True              # debug: gather pad slots instead of -1 trim


# ---------------------------------------------------------------- planning

class _P:
    pass


def _plan(edge_index, n_nodes, lo_lim=LO_LIM):
    ndst = n_nodes // NCORES
    nblk = (ndst + BLK - 1) // BLK
    src = np.concatenate([edge_index[0], np.arange(n_nodes)]).astype(np.int64)
    dst = np.concatenate([edge_index[1], np.arange(n_nodes)]).astype(np.int64)
    owner = dst // ndst
    per_core = []
    cnt = np.zeros((NCORES, nblk, 2), np.int64)
    for k in range(NCORES):
        m = owner == k
        s_k = src[m]
        d_k = dst[m] - k * ndst
        b_k = d_k // BLK
        j_k = d_k % BLK
        kind = (s_k >= lo_lim).astype(np.int64)   # 0 = lo, 1 = hi
        order = np.lexsort((s_k, kind, b_k))
        s_k, b_k, j_k, kind = s_k[order], b_k[order], j_k[order], kind[order]
        per_core.append((s_k, b_k, j_k, kind))
        np.add.at(cnt, (k, b_k, kind), 1)
    T_lo = np.maximum(1, -(-cnt[:, :, 0].max(axis=0) // TILE))
    T_hi = np.maximum(1, -(-cnt[:, :, 1].max(axis=0) // TILE))
    T_all = T_lo + T_hi
    off = np.zeros(nblk + 1, np.int64)
    off[1:] = np.cumsum(T_all)
    TT = int(off[-1])

    pl = _P()
    pl.ndst, pl.nblk, pl.n_nodes, pl.lo_lim = ndst, nblk, n_nodes, lo_lim
    pl.T_lo, pl.T_hi, pl.off, pl.TT = T_lo, T_hi, off, TT
    pl.cores = []
    for k in range(NCORES):
        s_k, b_k, j_k, kind = per_core[k]
        slot_src = np.full(TT * TILE, -1, np.int64)
        slot_j = np.full(TT * TILE, 255, np.int64)
        # per (block, kind) segment: real edges first, then -1 pads
        seg_id = b_k * 2 + kind
        bounds = np.searchsorted(seg_id, np.arange(2 * nblk + 1))
        for b in range(nblk):
            for kd in range(2):
                lo_i, hi_i = bounds[b * 2 + kd], bounds[b * 2 + kd + 1]
                n = hi_i - lo_i
                if n == 0:
                    continue
                s0 = (off[b] + (0 if kd == 0 else T_lo[b])) * TILE
                vals = s_k[lo_i:hi_i] - (lo_lim if kd else 0)
                slot_src[s0:s0 + n] = vals
                slot_j[s0:s0 + n] = j_k[lo_i:hi_i]
        # no gather call may trim to zero descriptors: give the first slot of
        # every call range a harmless real index (j stays 255 -> no effect)
        for b in range(nblk):
            for kd in range(2):
                T = (T_lo if kd == 0 else T_hi)[b]
                s0 = (off[b] + (0 if kd == 0 else T_lo[b])) * TILE
                for t0 in range(0, int(T), GCAP):
                    p = s0 + t0 * TILE
                    if slot_src[p] < 0:
                        slot_src[p] = 0
        if NO_TRIM:
            slot_src[slot_src < 0] = 0
        # per-call valid-prefix counts (gather trims trailing -1 to this)
        cnts = []
        for b in range(nblk):
            for kd in range(2):
                T = (T_lo if kd == 0 else T_hi)[b]
                s0 = (off[b] + (0 if kd == 0 else T_lo[b])) * TILE
                for t0 in range(0, int(T), GCAP):
                    t1 = min(t0 + GCAP, int(T))
                    seg = slot_src[s0 + t0 * TILE:s0 + t1 * TILE]
                    valid = np.nonzero(seg >= 0)[0]
                    cnts.append(int(valid[-1]) + 1 if len(valid) else 0)
        cp = _P()
        cp.slot_src, cp.slot_j = slot_src, slot_j
        cp.call_cnt = np.array(cnts, np.int32)
        pl.cores.append(cp)
    pl.ncalls = len(pl.cores[0].call_cnt)
    return pl


def _build_streams(pl, k):
    cp = pl.cores[k]
    TT = pl.TT
    # idx stream, wrapped [16, TT*8] then replicated to 8 Q7 groups
    idx = np.tile(cp.slot_src.astype(np.int16).reshape(TT * 8, 16).T, (8, 1))
    jst = cp.slot_j.reshape(TT, TILE).T.astype(np.float32)     # [128, TT]
    cnt = np.tile(cp.call_cnt[None, :], (128, 1))              # [128, ncalls]
    return np.ascontiguousarray(idx), np.ascontiguousarray(jst), \
        np.ascontiguousarray(cnt)


# ---------------------------------------------------------------- program

def build_program(pl, want_debug=False):
    import concourse.bass as bass
    import concourse.bacc as bacc
    import concourse.tile as tile
    import concourse.mybir as mybir

    F32 = mybir.dt.float32
    BF16 = mybir.dt.bfloat16
    I16 = mybir.dt.int16
    AF = mybir.ActivationFunctionType
    ALU = mybir.AluOpType

    n_nodes = pl.n_nodes
    ndst, nblk = pl.ndst, pl.nblk
    lo_lim = pl.lo_lim
    hi_rows = n_nodes - lo_lim
    T_lo, T_hi, off, TT = pl.T_lo, pl.T_hi, pl.off, pl.TT
    TLO_MX = int(T_lo.max())
    THI_MX = int(T_hi.max())
    TAL_MX = int((T_lo + T_hi).max())
    NPAD = nblk * BLK

    nc = bacc.Bacc("TRN2", target_bir_lowering=False, debug=want_debug,
                   num_devices=NCORES, num_swdge_queues=4,
                   dynamic_dma_scratch_size=32768)
    qrr = [0]

    def next_q():
        q = qrr[0] % 4
        qrr[0] += 1
        return q

    # -------- inputs
    xT = nc.dram_tensor("xT", [128, n_nodes], BF16, kind="ExternalInput")
    xoT = nc.dram_tensor("xoT", [128, NPAD], BF16, kind="ExternalInput")
    W1e = nc.dram_tensor("W1e", [128, F1 + HEADS], BF16, kind="ExternalInput")
    Vd1 = nc.dram_tensor("Vd1", [128, HEADS], BF16, kind="ExternalInput")
    WV2 = nc.dram_tensor("WV2", [128, 2, 18], BF16, kind="ExternalInput")
    IDENT = nc.dram_tensor("IDENT", [128, 128], BF16, kind="ExternalInput")
    IOTA = nc.dram_tensor("IOTA", [128, 128], F32, kind="ExternalInput")
    B1R = nc.dram_tensor("B1R", [128, F1], BF16, kind="ExternalInput")
    B2R = nc.dram_tensor("B2R", [128, OUT_CH], F32, kind="ExternalInput")
    RECB = nc.dram_tensor("RECB", [128, 18], F32, kind="ExternalInput")
    IDX = nc.dram_tensor("IDX", [128, TT * 8], I16, kind="ExternalInput")
    JST = nc.dram_tensor("JST", [128, TT], F32, kind="ExternalInput")
    OUT = nc.dram_tensor("OUT", [ndst, OUT_CH], F32, kind="ExternalOutput")

    NCH = -(-n_nodes // 128)
    BIGC = 8

    with tile.TileContext(nc) as tc:
        with (
            tc.tile_pool(name="dram", bufs=1, space="DRAM") as dpool,
            tc.tile_pool(name="const", bufs=1) as cpool,
            tc.tile_pool(name="persist", bufs=1) as ppool,
            tc.tile_pool(name="node", bufs=3) as npool,
            tc.tile_pool(name="edge", bufs=3) as epool,
            tc.tile_pool(name="sel", bufs=2) as selpool,
            tc.tile_pool(name="small", bufs=3) as spool,
            tc.tile_pool(name="ps_e", bufs=2, space="PSUM") as ps_e,
            tc.tile_pool(name="ps_a", bufs=2, space="PSUM") as ps_a,
            tc.tile_pool(name="ps_m", bufs=2, space="PSUM") as ps_m,
            tc.tile_pool(name="ps_n", bufs=2, space="PSUM") as ps_n,
        ):
            R1 = dpool.tile([n_nodes, R1_W], BF16)
            REC = dpool.tile([ndst, REC_W], F32)
            R2 = dpool.tile([n_nodes, REC_W], F32, addr_space="Shared")

            # consts
            cW1e = cpool.tile([128, F1 + HEADS], BF16)
            cVd1 = cpool.tile([128, HEADS], BF16)
            cWV2 = cpool.tile([128, 2, 18], BF16)
            cID = cpool.tile([128, 128], BF16)
            cIOTA = cpool.tile([128, 128], F32)
            cB1 = cpool.tile([128, F1], BF16)
            cB2 = cpool.tile([128, OUT_CH], F32)
            cRB = cpool.tile([128, 18], F32)
            cZ = cpool.tile([128, F1], BF16)
            nc.vector.memset(cZ[:], 0.0)
            nc.sync.dma_start(cW1e[:], W1e[:])
            nc.sync.dma_start(cVd1[:], Vd1[:])
            nc.sync.dma_start(cWV2[:], WV2[:])
            nc.sync.dma_start(cID[:], IDENT[:])
            nc.sync.dma_start(cIOTA[:], IOTA[:])
            nc.sync.dma_start(cB1[:], B1R[:])
            nc.sync.dma_start(cB2[:], B2R[:])
            nc.sync.dma_start(cRB[:], RECB[:])

            cIDX = ppool.tile([128, TT * 8], I16)
            cJ = ppool.tile([128, TT], F32)
            nc.sync.dma_start(cIDX[:], IDX[:])
            nc.sync.dma_start(cJ[:], JST[:])

            alD1 = ppool.tile([128, nblk, HEADS], BF16)
            alD2 = ppool.tile([128, nblk, 1], BF16)

            # ---------------- N1: R1 node table (bf16 records)
            for c0 in range(0, NCH, BIGC):
                c1 = min(c0 + BIGC, NCH)
                ncols = min(n_nodes - c0 * 128, BIGC * 128)
                xt = npool.tile([128, BIGC * 128], BF16, tag="xt")
                nc.sync.dma_start(xt[:, 0:ncols], xT[:, c0 * 128:c0 * 128 + ncols])
                rw = npool.tile([128, BIGC, F1 + HEADS], BF16, tag="rw")
                for c in range(c0, c1):
                    m = min(128, n_nodes - c * 128)
                    pn = ps_n.tile([128, F1 + HEADS], F32, tag="pn",
                                   padded_shape=[128, 512])
                    nc.tensor.matmul(pn[0:m, :],
                                     xt[:, (c - c0) * 128:(c - c0) * 128 + m],
                                     cW1e[:], start=True, stop=True)
                    nc.scalar.copy(rw[0:m, c - c0, :], pn[0:m, :])
                nfull = (min(n_nodes, c1 * 128) - c0 * 128) // 128
                if nfull:
                    nc.sync.dma_start(
                        R1[c0 * 128:c0 * 128 + nfull * 128, 0:F1 + HEADS]
                        .rearrange("(g p) e -> p g e", p=128),
                        rw[:, 0:nfull, :])
                rem = min(n_nodes, c1 * 128) - c0 * 128 - nfull * 128
                if rem:
                    nc.sync.dma_start(
                        R1[c0 * 128 + nfull * 128:min(n_nodes, c1 * 128),
                           0:F1 + HEADS]
                        .rearrange("(g p) e -> p g e", p=rem),
                        rw[0:rem, nfull:nfull + 1, :])

            # ---------------- N2: own al_dst1
            xo = ppool.tile([128, NPAD], BF16)
            nc.sync.dma_start(xo[:], xoT[:])
            for b in range(nblk):
                po = ps_n.tile([128, HEADS], F32, tag="pn",
                               padded_shape=[128, 512])
                nc.tensor.matmul(po[:], xo[:, b * 128:(b + 1) * 128], cVd1[:],
                                 start=True, stop=True)
                nc.scalar.copy(alD1[:, b, :], po[:])

            # ---------------- shared edge-stage builder
            def edge_stage(layer, post_block=None):
                if layer == 1:
                    EW, EDT = R1_W, BF16
                    F, H = F1, HEADS
                    tbl_lo = R1[0:lo_lim, :]
                    tbl_hi = R1[lo_lim:n_nodes, :]
                    alD = alD1
                else:
                    EW, EDT = REC_W, F32
                    F, H = OUT_CH, 1
                    tbl_lo = R2[0:lo_lim, :]
                    tbl_hi = R2[lo_lim:n_nodes, :]
                    alD = alD2
                for b in range(nblk):
                    tl, th = int(T_lo[b]), int(T_hi[b])
                    ta = tl + th
                    ob = int(off[b])
                    gl = epool.tile([128, TLO_MX, EW], EDT, tag=f"gl{layer}")
                    gh = epool.tile([128, THI_MX, EW], EDT, tag=f"gh{layer}")
                    if b < 2:
                        nc.vector.memset(gl[:], 0.0)
                        nc.vector.memset(gh[:], 0.0)
                    for t0 in range(0, tl, GCAP):
                        t1 = min(t0 + GCAP, tl)
                        ni = (t1 - t0) * TILE
                        nc.gpsimd.dma_gather(
                            gl[:, t0:t1, :], tbl_lo,
                            cIDX[:, (ob + t0) * 8:(ob + t1) * 8], ni, ni,
                            EW, queue_num=next_q())
                    for t0 in range(0, th, GCAP):
                        t1 = min(t0 + GCAP, th)
                        ni = (t1 - t0) * TILE
                        nc.gpsimd.dma_gather(
                            gh[:, t0:t1, :], tbl_hi,
                            cIDX[:, (ob + tl + t0) * 8:(ob + tl + t1) * 8],
                            ni, ni, EW, queue_num=next_q())

                    # layer 2: compact gathered records to bf16 [slot, t, 18]
                    if layer == 2:
                        g2b = spool.tile([128, TAL_MX, 18], BF16, tag="g2b")
                        nc.vector.tensor_copy(g2b[:, 0:tl, :], gl[:, 0:tl, 0:18])
                        nc.vector.tensor_copy(g2b[:, tl:ta, :], gh[:, 0:th, 0:18])

                    # S tiles: one-hot [slot, t, dst] built on-chip
                    sS = selpool.tile([128, TAL_MX, 128], BF16, tag="sS")
                    nc.vector.tensor_tensor(
                        sS[:, 0:ta, :],
                        cJ[:, ob:ob + ta].unsqueeze(2).broadcast_to([128, ta, 128]),
                        cIOTA[:].unsqueeze(1).broadcast_to([128, ta, 128]),
                        op=ALU.is_equal)

                    # batched: 8x ST = S^T on PE -> one ACT copy -> expansions
                    pe = ps_e.tile([128, TAL_MX * HEADS], F32, tag="pe",
                                   padded_shape=[128, 512])
                    for tb in range(0, ta, 8):
                        te = min(tb + 8, ta)
                        ptr = ps_n.tile([128, 8, 128], BF16, tag="pn",
                                        padded_shape=[128, 8, 128])
                        for ti in range(tb, te):
                            nc.tensor.transpose(ptr[:, ti - tb, :],
                                                sS[:, ti, :], cID[:])
                        sTb = spool.tile([128, 8, 128], BF16, tag="sT")
                        nc.scalar.copy(sTb[:, 0:te - tb, :],
                                       ptr[:, 0:te - tb, :])
                        for ti in range(tb, te):
                            nc.tensor.matmul(pe[:, ti * H:(ti + 1) * H],
                                             sTb[:, ti - tb, :],
                                             alD[:, b, :],
                                             start=True, stop=True)
                    eD = spool.tile([128, TAL_MX * HEADS], BF16, tag="eD")
                    nc.scalar.copy(eD[:, 0:ta * H], pe[:, 0:ta * H])

                    # e = al_src + e_dst ; lrelu ; exp ; weight messages
                    def gv(lo):
                        if layer == 2:
                            return g2b[:, 0:tl, :] if lo else g2b[:, tl:ta, :]
                        return gl[:, 0:tl, :] if lo else gh[:, 0:th, :]

                    exw = selpool.tile([128, TAL_MX, F], BF16,
                                       tag=f"exw{layer}")
                    for lo in (True, False):
                        t = gv(lo)
                        tn = tl if lo else th
                        eoff = 0 if lo else tl * H
                        toff = 0 if lo else tl
                        ecols = t[:, :, F:F + H]
                        nc.vector.tensor_add(
                            ecols, ecols,
                            eD[:, eoff:eoff + tn * H].rearrange(
                                "p (t h) -> p t h", h=H))
                        nc.vector.scalar_tensor_tensor(
                            ecols, ecols, NEG_SLOPE, ecols,
                            op0=ALU.mult, op1=ALU.max)
                        nc.scalar.activation(ecols, ecols, AF.Exp)
                        C = F // H
                        # expand per-(slot,head) weights to F cols on ACT so
                        # the DVE multiply below runs contiguous at 2x rate
                        nc.scalar.copy(
                            exw[:, toff:toff + tn, :].rearrange(
                                "p t (h c) -> p t h c", c=C),
                            ecols.unsqueeze(3).broadcast_to([128, tn, H, C]))
                        nc.vector.tensor_mul(
                            t[:, :, 0:F], t[:, :, 0:F],
                            exw[:, toff:toff + tn, :])

                    # aggregation: psum [dst, F+H] accumulated over tiles
                    pa = ps_a.tile([128, F + H], F32, tag="pa",
                                   padded_shape=[128, 512])
                    for ti in range(ta):
                        lo = ti < tl
                        i = ti if lo else ti - tl
                        rhs = gv(lo)[:, i, 0:F + H]
                        nc.tensor.matmul(pa[:], sS[:, ti, :], rhs,
                                         start=(ti == 0), stop=(ti == ta - 1))

                    # finalize
                    den = spool.tile([128, H], F32, tag="den")
                    nc.scalar.activation(den[:], pa[:, F:F + H], AF.Copy,
                                         bias=DEN_EPS)
                    rden = spool.tile([128, H], F32, tag="rden")
                    nc.vector.reciprocal(rden[:], den[:])
                    m = min(BLK, ndst - b * BLK)
                    if layer == 1:
                        C = F // H
                        h2t = spool.tile([128, F], BF16, tag="h2t")
                        nc.vector.tensor_mul(
                            h2t[:].rearrange("p (h c) -> p h c", c=C),
                            pa[:, 0:F].rearrange("p (h c) -> p h c", c=C),
                            rden[:].unsqueeze(2).broadcast_to([128, H, C]))
                        nc.vector.tensor_add(h2t[:], h2t[:], cB1[:])
                        # ELU without the -1 (folded into record bias)
                        t1 = spool.tile([128, F], BF16, tag="t1")
                        nc.vector.tensor_tensor(t1[:], h2t[:], cZ[:],
                                                op=ALU.min)
                        nc.scalar.activation(t1[:], t1[:], AF.Exp)
                        nc.vector.tensor_tensor(h2t[:], h2t[:], cZ[:],
                                                op=ALU.max)
                        nc.vector.tensor_add(h2t[:], h2t[:], t1[:])
                        ptr = ps_m.tile([128, 2, 128], BF16, tag="m",
                                        padded_shape=[128, 2, 128])
                        nc.tensor.transpose(ptr[:, 0, :], h2t[:, 0:128], cID[:])
                        nc.tensor.transpose(ptr[:, 1, :], h2t[:, 128:256], cID[:])
                        h2T = spool.tile([128, 2, 128], BF16, tag="h2T")
                        nc.scalar.copy(h2T[:], ptr[:])
                        prc = ps_m.tile([128, 18], F32, tag="m",
                                        padded_shape=[128, 512])
                        nc.tensor.matmul(prc[:], h2T[:, 0, :], cWV2[:, 0, :],
                                         start=True, stop=False)
                        nc.tensor.matmul(prc[:], h2T[:, 1, :], cWV2[:, 1, :],
                                         start=False, stop=True)
                        rec = spool.tile([128, 18], F32, tag="rec")
                        nc.vector.tensor_add(rec[:], prc[:], cRB[:])
                        nc.scalar.copy(alD2[:, b, :], rec[:, 17:18])
                        nc.sync.dma_start(REC[b * BLK:b * BLK + m, 0:18],
                                          rec[0:m, :])
                    else:
                        v = spool.tile([128, OUT_CH], F32, tag="v")
                        nc.vector.tensor_mul(
                            v[:], pa[:, 0:OUT_CH],
                            rden[:].broadcast_to([128, OUT_CH]))
                        nc.vector.tensor_add(v[:], v[:], cB2[:])
                        mx = spool.tile([128, 1], F32, tag="mx")
                        nc.vector.tensor_reduce(mx[:], v[:], op=ALU.max,
                                                axis=mybir.AxisListType.X)
                        nc.vector.tensor_sub(
                            v[:], v[:], mx[:].broadcast_to([128, OUT_CH]))
                        ex = spool.tile([128, OUT_CH], F32, tag="exf")
                        sm = spool.tile([128, 1], F32, tag="sm")
                        nc.scalar.activation(ex[:], v[:], AF.Exp,
                                             accum_out=sm[:])
                        lns = spool.tile([128, 1], F32, tag="lns")
                        nc.scalar.activation(lns[:], sm[:], AF.Ln)
                        nc.vector.tensor_sub(
                            v[:], v[:], lns[:].broadcast_to([128, OUT_CH]))
                        nc.sync.dma_start(OUT[b * BLK:b * BLK + m, :],
                                          v[0:m, :])
                    if post_block is not None:
                        post_block(b)

            # ---------------- E1, AllGather, E2
            edge_stage(1)
            nc.gpsimd.collective_compute(
                "AllGather", mybir.AluOpType.bypass,
                replica_groups=[list(range(NCORES))],
                ins=[REC.opt()], outs=[R2.opt()])
            edge_stage(2)

    nc.compile()
    return nc


# ---------------------------------------------------------------- host prep

def _host_inputs(pl, inputs):
    x = np.asarray(inputs["x"], np.float32)
    W1 = np.asarray(inputs["W1"], np.float32)
    a_s1 = np.asarray(inputs["a_src1"], np.float32)
    a_d1 = np.asarray(inputs["a_dst1"], np.float32)
    b1 = np.asarray(inputs["b1"], np.float32)
    W2 = np.asarray(inputs["W2"], np.float32)
    a_s2 = np.asarray(inputs["a_src2"], np.float32)
    a_d2 = np.asarray(inputs["a_dst2"], np.float32)
    b2 = np.asarray(inputs["b2"], np.float32)
    n_nodes, ndst, nblk = pl.n_nodes, pl.ndst, pl.nblk
    NPAD = nblk * BLK

    A_s1 = np.zeros((F1, HEADS), np.float32)
    A_d1 = np.zeros((F1, HEADS), np.float32)
    for h in range(HEADS):
        A_s1[h * HID:(h + 1) * HID, h] = a_s1[h]
        A_d1[h * HID:(h + 1) * HID, h] = a_d1[h]
    V_s1 = (W1 @ A_s1).astype(np.float32)
    V_d1 = (W1 @ A_d1).astype(np.float32)
    V_s2 = (W2 @ a_s2[0]).astype(np.float32)
    V_d2 = (W2 @ a_d2[0]).astype(np.float32)
    WV2 = np.concatenate([W2, V_s2[:, None], V_d2[:, None]], axis=1)  # [256,18]
    RECB = -WV2.sum(axis=0, keepdims=True)          # [1, 18] (the ELU -1 fold)

    import ml_dtypes
    BF = ml_dtypes.bfloat16
    iota = np.tile(np.arange(128, dtype=np.float32)[None, :], (128, 1))
    common = {
        "xT": np.ascontiguousarray(x.T.astype(BF)),
        "W1e": np.ascontiguousarray(
            np.concatenate([W1, V_s1], axis=1).astype(BF)),
        "Vd1": np.ascontiguousarray(V_d1.astype(BF)),
        "WV2": np.ascontiguousarray(
            WV2.reshape(2, 128, 18).transpose(1, 0, 2).astype(BF)),
        "IDENT": np.eye(128, dtype=np.float32).astype(BF),
        "IOTA": np.ascontiguousarray(iota),
        "B1R": np.tile(b1[None, :], (128, 1)).astype(BF),
        "B2R": np.tile(b2[None, :], (128, 1)).astype(np.float32),
        "RECB": np.tile(RECB, (128, 1)).astype(np.float32),
    }
    in_maps = []
    for k in range(NCORES):
        idx, jst, cnt = _build_streams(pl, k)
        xo = np.zeros((NPAD, 128), np.float32)
        xo[0:ndst] = x[k * ndst:(k + 1) * ndst]
        m = dict(common)
        m["xoT"] = np.ascontiguousarray(xo.T.astype(BF))
        m["IDX"] = idx
        m["JST"] = jst
        in_maps.append(m)
    return in_maps


# ---------------------------------------------------------------- entry

def _run(inputs, trace=False, **kw):
    from concourse.bass_utils import run_bass_kernel_spmd

    edge_index = np.asarray(inputs["edge_index"])
    n_nodes = int(np.asarray(inputs["x"]).shape[0])
    pl = _plan(edge_index, n_nodes)
    nc = build_program(pl)
    in_maps = _host_inputs(pl, inputs)
    res = run_bass_kernel_spmd(nc, in_maps, list(range(NCORES)),
                               trace=trace, **kw)
    out = np.concatenate([res.results[k]["OUT"] for k in range(NCORES)], axis=0)
    return out.astype(np.float32), res


def kernel(**inputs):
    out, _ = _run(inputs)
    return out
